# revision 1
# baseline (speedup 1.0000x reference)
"""Trainium2 Bass kernel for the dynamic-filter CNN (DCM) module.

Reference computation (per sample b):
  pooled    = adaptive_avg_pool2d(x[b], (3,3))                  # [Cin,3,3]
  gen_filt  = filter_gen_w @ pooled + filter_gen_b              # [C,3,3]
  xr        = relu(redu_w @ x[b] + redu_b)                      # [C,H,W]
  dw        = relu(depthwise3x3(xr, gen_filt, zero-pad 1))      # [C,H,W]
  out       = relu(fusion_w @ dw + fusion_b)                    # [C,H,W]

Sharding: 8 cores = (batch 4) x (H-half 2). Each core owns 32 output rows and
loads one halo row each side. Bottom-half cores receive their rows REVERSED by
the host so a single SPMD graph works for all cores; the 3x3 filter is
mirrored per-core and the adaptive-pool bin placement resolved per-core via
tiny host-supplied 0/1 mask tensors + a pair-wise AllReduce.

Shapes hardcoded for x=[4,2048,64,64] f32, C=512.
"""
import os
import numpy as np
import ml_dtypes

import concourse.bass as bass
import concourse.mybir as mybir
import concourse.tile as tile
from concourse.bass_utils import run_bass_kernel_spmd
from concourse.vector_clock import ScopedClock

F32 = mybir.dt.float32
BF16 = mybir.dt.bfloat16


# Workaround for this container's walrus codegen: an instruction's inline sync
# header only supports one wait command ("Too many sync wait commands" in
# CoreV3GenImpl setupSyncWait), but Tile's kernel-tail drain attaches one wait
# per logical proc. Spread the drain's waits across preceding nofuse NOPs on
# the same engine (program order keeps the drain after all of them).
def _patched_drain_and_barrier(self, tick_clock, wait_clock):
    nops = [self.nc.sync.nop(nofuse=True, hint="drain_wait_spread")
            for _ in range(28)]
    drain_inst = self.nc.sync.drain()
    wait_clock.add_sem_waits(
        drain_inst.ins, ScopedClock({None: tick_clock.global_clock}))
    si = drain_inst.ins.sync_info
    waits = list(si.on_wait) if si is not None and si.on_wait else []
    if len(waits) > 1:
        assert len(waits) <= len(nops) + 1, f"too many drain waits: {len(waits)}"
        for i, wentry in enumerate(waits[1:]):
            nops[i].ins.sync_info = mybir.SyncInfo(
                on_wait=[wentry], on_update=[])
        drain_inst.ins.sync_info = mybir.SyncInfo(
            on_wait=[waits[0]], on_update=list(si.on_update or []))
    self.nc.all_engine_barrier()
    popped = self.nc._tile_sem_poison_stack.pop()
    assert popped is self._sem_poison
    self.nc.clear_and_free_semaphores(list(self.sems.allocated().values()))
    self.nc.all_engine_barrier()


tile.TileContext._drain_and_barrier = _patched_drain_and_barrier



def _dedup_ldweights(nc):
    """Tile lowering splits every matmul into Ldweights+Matmult; with walrus
    ldw-opt disabled each pair reloads the stationary operand even when
    consecutive matmuls share it. Replace redundant consecutive Ldweights
    (same weights AP + tile params, only Matmults in between on PE) with
    NoOps that keep their sync_info."""
    n_removed = 0
    for f in nc.m.functions:
        for bb in f.blocks:
            last_key = None
            insts = bb.instructions
            for idx, inst in enumerate(insts):
                tname = type(inst).__name__
                if tname == "InstLdweights":
                    key = (
                        str(inst.ins[0]),
                        str(getattr(inst, "tile_position", None)),
                        str(getattr(inst, "tile_size", None)),
                        str(getattr(inst, "perf_mode", None)),
                        str(getattr(inst, "is_transpose", None)),
                    )
                    if key == last_key:
                        nop = mybir.InstNoOp(
                            name=f"I-ldwdedup-{n_removed}", ins=[], outs=[])
                        nop.engine = inst.engine
                        nop.sync_info = inst.sync_info
                        insts[idx] = nop
                        n_removed += 1
                    else:
                        last_key = key
                elif tname == "InstMatmult" or inst.engine != mybir.EngineType.PE:
                    continue
                else:
                    last_key = None
    return n_removed


def _split_multiwait_instructions(nc):
    """Same walrus limitation, applied generically: any instruction whose
    sync header carries >1 wait gets its extra waits moved onto NoOps
    inserted just before it on the same engine (per-engine order is the
    block-list order filtered by engine, so this preserves semantics)."""
    ctr = [0]
    for f in nc.m.functions:
        for bb in f.blocks:
            insts = bb.instructions
            out = []
            for inst in insts:
                si = getattr(inst, "sync_info", None)
                waits = list(si.on_wait) if si is not None and si.on_wait else []
                if len(waits) > 1:
                    for w in waits[:-1]:
                        nop = mybir.InstNoOp(
                            name=f"I-waitsplit-{ctr[0]}", ins=[], outs=[])
                        ctr[0] += 1
                        nop.engine = inst.engine
                        nop.sync_info = mybir.SyncInfo(
                            on_wait=[w], on_update=[])
                        out.append(nop)
                    inst.sync_info = mybir.SyncInfo(
                        on_wait=[waits[-1]],
                        on_update=list(si.on_update or []))
                out.append(inst)
            if len(out) != len(insts):
                insts[:] = out

CIN = 2048
C = 512
H = 64
W = 64
KT = CIN // 128   # 16 cin tiles
MT = C // 128     # 4 cout tiles
ROWS = 34         # row 0 = edge pad (zeros from host), 1..32 owned, 33 = halo
WPAD = 68         # xr pad layout: data cols 2..65; taps read cols 1..66

# local pool row bins (uniform on every core thanks to row reversal):
#   L0 = rows 0..22 (incl. zero pad row -> contributes 0), L1 = rows 22..32
# w bins of adaptive pool 64->3: [0,22), [21,43), [42,64)
WBINS = [(0, 22), (21, 43), (42, 64)]
ROW_BLOCKS = [(1, 9), (9, 17), (17, 25), (25, 33)]  # xr rows (halo deferred)
OUT_BLOCKS = [(0, 8), (8, 16), (16, 24), (24, 32)]            # output rows

_CACHE = {}


def _l1_bins_view(t):
    """[128, 3(q), 11(rows 22..32), 22(w)] overlapping-bin view of an
    [128, 34, 64] tile: w-bin starts {0, 21, 42} (step 21)."""
    import bass_rust
    v = t[:].copy()
    v.ap = bass_rust.VecI64Pair([[34 * 64, 128], [21, 3], [64, 11], [1, 22]])
    v.offset = 22 * 64
    return v


def build_graph():
    nc = bass.Bass(num_devices=8)

    x_in = nc.declare_dram_parameter("x_sh", [CIN, ROWS, W], F32, isOutput=False)
    reduT_d = nc.declare_dram_parameter("reduT", [CIN, C], BF16, isOutput=False)
    fgwT_d = nc.declare_dram_parameter("fgwT", [CIN, C], BF16, isOutput=False)
    fuT_d = nc.declare_dram_parameter("fuT", [C, C], BF16, isOutput=False)
    # blob layout: rb[0:4] gb[4:8] fb[8:12] maskgf[12:30] mask9[30:174]
    blob_d = nc.declare_dram_parameter("blob", [128, 174], F32, isOutput=False)
    eye_d = nc.declare_dram_parameter("eye", [128, 128], BF16, isOutput=False)
    out_d = nc.declare_dram_parameter("out", [C, 32, W], F32, isOutput=True)

    pool_part = nc.dram_tensor("pool_part", [CIN, 9], F32)
    pool_red = nc.dram_tensor("pool_red", [CIN, 9], F32)

    AF = mybir.ActivationFunctionType
    OP = mybir.AluOpType

    with tile.TileContext(nc) as tc:
        with (
            tc.tile_pool(name="const", bufs=1) as const,
            tc.tile_pool(name="xf", bufs=3) as xfp,
            tc.tile_pool(name="work", bufs=2) as work,
            tc.tile_pool(name="dw", bufs=1) as dwp,
            tc.tile_pool(name="osb", bufs=4) as osbp,
            tc.tile_pool(name="ps", bufs=8, space="PSUM") as ps,
        ):
            # ---- x load first (PE's critical path), weights batched ----
            pool_acc = const.tile([128, KT, 6], F32, tag="pacc")
            xbf = []
            for k in range(KT):
                xbf.append(const.tile([128, ROWS, W], BF16, tag=f"xbf{k}", name=f"xbf{k}"))
            blob_sb = const.tile([128, 174], F32, tag="blob")
            nc.sync.dma_start(blob_sb[:], blob_d[:])
            eye_sb = const.tile([128, 128], BF16, tag="eye")
            nc.sync.dma_start(eye_sb[:], eye_d[:])
            rb_sb = blob_sb[:, 0:4]
            gb_sb = blob_sb[:, 4:8]
            fb_sb = blob_sb[:, 8:12]
            maskgf_sb = blob_sb[:, 12:30]
            mask9_sb = blob_sb[:, 30:174].rearrange("p (k q) -> p k q", q=9)
            reduT_sb = const.tile([128, KT, C], BF16, tag="reduT")
            fuT_sb = const.tile([128, MT, C], BF16, tag="fuT")
            fgwT_sb = const.tile([128, KT, C], BF16, tag="fgwT")
            xfs = []
            for k in range(KT):
                xf = xfp.tile([128, ROWS, W], F32, tag="xf", name="xf")
                nc.sync.dma_start(xf[:], x_in[k * 128:(k + 1) * 128, :, :])
                nc.vector.tensor_copy(xbf[k][:], xf[:])
                xfs.append(xf)
                if k == 0:
                    nc.sync.dma_start(
                        reduT_sb[:],
                        reduT_d.rearrange("(k p) c -> p k c", p=128))
                elif k == 8:
                    nc.sync.dma_start(
                        fgwT_sb[:],
                        fgwT_d.rearrange("(k p) c -> p k c", p=128))
                elif k == 10:
                    nc.sync.dma_start(
                        fuT_sb[:], fuT_d.rearrange("(k p) c -> p k c", p=128))
                # pool partials: L0 rows (3 bins) on ACT via accum_out, L1
                # rows (3 bins) on DVE via one XY tensor_reduce
                for q, (w0, w1) in enumerate(WBINS):
                    pdump = work.tile([128, 23, 22], BF16, tag="pdump",
                                      name="pdump")
                    nc.scalar.activation(
                        out=pdump[:, 0:23, :],
                        in_=xbf[k][:, 0:23, w0:w1],
                        func=AF.Copy,
                        accum_out=pool_acc[:, k, q:q + 1],
                    )
                nc.vector.tensor_reduce(
                    out=pool_acc[:, k, 3:6],
                    in_=_l1_bins_view(xbf[k]),
                    axis=mybir.AxisListType.XY,
                    op=OP.add,
                )

            # ---- reduction conv: xr = relu(reduT.T @ x + rb), bf16, padded ----
            xr = []
            for m in range(MT):
                t = const.tile([128, ROWS, WPAD], BF16, tag=f"xr{m}", name=f"xr{m}")
                xr.append(t)
                nc.gpsimd.memset(t[:, 0:1, :], 0.0)        # edge pad row
                nc.gpsimd.memset(t[:, :, 1:2], 0.0)        # left pad col (w=-1)
                nc.gpsimd.memset(t[:, :, 66:67], 0.0)      # right pad col (w=64)
            # Interleave m in pairs (2 m x 4 row blocks = 8 PSUM banks) so
            # half the output channels accumulate in lockstep with the x
            # stream; the 1-row halo block runs as a cheap post-load pass.
            for pair in ((0, 1), (2, 3)):
                pst = {m: [ps.tile([128, 8, W], F32, tag="ps",
                                   name=f"psr{m}_{bi}")
                           for bi in range(len(ROW_BLOCKS))] for m in pair}
                for k in range(KT):
                    for m in pair:
                        for bi, (r0, r1) in enumerate(ROW_BLOCKS):
                            nc.tensor.matmul(
                                pst[m][bi][:],
                                reduT_sb[:, k, m * 128:(m + 1) * 128],
                                xbf[k][:, r0:r1, :],
                                start=(k == 0), stop=(k == KT - 1),
                            )
                for m in pair:
                    for bi, (r0, r1) in enumerate(ROW_BLOCKS):
                        nc.vector.tensor_scalar(
                            out=xr[m][:, r0:r1, 2:66],
                            in0=pst[m][bi][:],
                            scalar1=rb_sb[:, m:m + 1],
                            scalar2=0.0,
                            op0=OP.add, op1=OP.max,
                        )
            for m in range(MT):
                ph = ps.tile([128, 1, W], F32, tag="ps", name=f"psh{m}")
                for k in range(KT):
                    nc.tensor.matmul(
                        ph[:],
                        reduT_sb[:, k, m * 128:(m + 1) * 128],
                        xbf[k][:, 33:34, :],
                        start=(k == 0), stop=(k == KT - 1),
                    )
                nc.vector.tensor_scalar(
                    out=xr[m][:, 33:34, 2:66],
                    in0=ph[:],
                    scalar1=rb_sb[:, m:m + 1],
                    scalar2=0.0,
                    op0=OP.add, op1=OP.max,
                )

            # ---- pooled scatter + AllReduce + gen matmul, chunked over k so
            # the collective and filter-gen pipeline behind the x load instead
            # of serializing after it (PSUM freed per chunk; accumulate the
            # chunk partials in SBUF) ----
            dup = work.tile([128, KT, 9], F32, tag="dup", bufs=1)
            scat = work.tile([128, KT, 9], F32, tag="scat", bufs=1)
            pooled_f = work.tile([128, KT, 9], F32, tag="poolf", bufs=1)
            pooled_bf = work.tile([128, KT, 9], BF16, tag="poolbf", bufs=1)
            gen_acc = work.tile([128, MT, 9], F32, tag="genacc", bufs=1)
            CH = 8
            for ch in range(KT // CH):
                k0, k1 = ch * CH, (ch + 1) * CH
                nc.vector.tensor_copy(dup[:, k0:k1, 0:6],
                                      pool_acc[:, k0:k1, 0:6])
                nc.vector.tensor_copy(dup[:, k0:k1, 6:9],
                                      pool_acc[:, k0:k1, 0:3])
                nc.vector.tensor_mul(scat[:, k0:k1, :], dup[:, k0:k1, :],
                                     mask9_sb[:, k0:k1, :])
                nc.sync.dma_start(
                    pool_part[k0 * 128:k1 * 128, :].rearrange(
                        "(k p) q -> p k q", p=128),
                    scat[:, k0:k1, :])
                nc.gpsimd.collective_compute(
                    "AllReduce",
                    OP.add,
                    replica_groups=[[0, 1], [2, 3], [4, 5], [6, 7]],
                    ins=[pool_part[k0 * 128:k1 * 128, :]],
                    outs=[pool_red[k0 * 128:k1 * 128, :]],
                )
                nc.sync.dma_start(
                    pooled_f[:, k0:k1, :],
                    pool_red[k0 * 128:k1 * 128, :].rearrange(
                        "(k p) q -> p k q", p=128))
                nc.vector.tensor_copy(pooled_bf[:, k0:k1, :],
                                      pooled_f[:, k0:k1, :])
                for m in range(MT):
                    pg = ps.tile([128, 16], F32, tag="ps",
                                 name=f"psg{m}_{ch}")
                    for k in range(k0, k1):
                        nc.tensor.matmul(
                            pg[:, 0:9],
                            fgwT_sb[:, k, m * 128:(m + 1) * 128],
                            pooled_bf[:, k, :],
                            start=(k == k0), stop=(k == k1 - 1),
                        )
                    if ch == 0:
                        nc.vector.tensor_copy(gen_acc[:, m, :], pg[:, 0:9])
                    else:
                        nc.vector.tensor_add(gen_acc[:, m, :],
                                             gen_acc[:, m, :], pg[:, 0:9])

            # ---- per-core mirror + diag tiles ----
            diag = [[None] * 9 for _ in range(MT)]
            for m in range(MT):
                gf = work.tile([128, 9], F32, tag="gf")
                nc.scalar.activation(
                    out=gf[:], in_=gen_acc[:, m, :], func=AF.Identity,
                    bias=gb_sb[:, m:m + 1])
                gfdup = work.tile([128, 18], F32, tag="gfdup")
                nc.vector.tensor_copy(gfdup[:, 0:9], gf[:])
                for dy in range(3):
                    nc.vector.tensor_copy(
                        gfdup[:, 9 + 3 * dy:12 + 3 * dy],
                        gf[:, 3 * (2 - dy):3 * (2 - dy) + 3])
                gft = work.tile([128, 18], F32, tag="gft")
                nc.vector.tensor_mul(gft[:], gfdup[:], maskgf_sb[:])
                gfu = const.tile([128, 9], F32, tag=f"gfu{m}", name=f"gfu{m}")
                nc.vector.tensor_add(gfu[:], gft[:, 0:9], gft[:, 9:18])
                for t in range(9):
                    d = const.tile([128, 128], BF16, tag=f"dg{m}_{t}", name=f"dg{m}_{t}")
                    nc.vector.tensor_scalar_mul(d[:], eye_sb[:], gfu[:, t:t + 1])
                    diag[m][t] = d

            # ---- depthwise 3x3: diagonal matmuls packed as 4 concurrent
            # 32x32 tile_position groups (the diag only links partition i to
            # output partition i, so each 32-block is independent) ----
            dw_bf = [[None] * len(OUT_BLOCKS) for _ in range(MT)]
            for m in range(MT):
                pdm = [ps.tile([128, 8, W], F32, tag="ps", name=f"psd{m}_{bi}")
                       for bi in range(len(OUT_BLOCKS))]
                for t in range(9):
                    dy, dx = t // 3, t % 3
                    for p in range(4):
                        pp = 32 * p
                        for bi, (o0, o1) in enumerate(OUT_BLOCKS):
                            nc.tensor.matmul(
                                pdm[bi][pp:pp + 32, :, :],
                                diag[m][t][pp:pp + 32, pp:pp + 32],
                                xr[m][pp:pp + 32, o0 + dy:o1 + dy,
                                      dx + 1:dx + 65],
                                start=(t == 0), stop=(t == 8),
                                tile_position=(pp, pp),
                            )
                for bi in range(len(OUT_BLOCKS)):
                    d = dwp.tile([128, 8, W], BF16, tag=f"dwbf{m}_{bi}",
                                 name=f"dwbf{m}_{bi}")
                    nc.scalar.activation(d[:], pdm[bi][:], AF.Relu)
                    dw_bf[m][bi] = d

            # ---- fusion conv, LDW amortized across row blocks ----
            for co in range(MT):
                pf = [ps.tile([128, 8, W], F32, tag="ps", name=f"psf{co}_{bi}")
                      for bi in range(len(OUT_BLOCKS))]
                for kc in range(MT):
                    for bi in range(len(OUT_BLOCKS)):
                        nc.tensor.matmul(
                            pf[bi][:],
                            fuT_sb[:, kc, co * 128:(co + 1) * 128],
                            dw_bf[kc][bi][:],
                            start=(kc == 0), stop=(kc == MT - 1),
                        )
                for bi, (o0, o1) in enumerate(OUT_BLOCKS):
                    osb = osbp.tile([128, 8, W], F32, tag="osb", name="osb")
                    nc.scalar.activation(
                        out=osb[:], in_=pf[bi][:], func=AF.Relu,
                        bias=fb_sb[:, co:co + 1])
                    nc.sync.dma_start(
                        out_d[co * 128:(co + 1) * 128, o0:o1, :], osb[:])
    _dedup_ldweights(nc)
    _split_multiwait_instructions(nc)
    return nc


def _host_inputs(x, filter_gen_w, filter_gen_b, redu_w, redu_b, fusion_w,
                 fusion_b):
    bf = ml_dtypes.bfloat16
    shared = {
        "reduT": np.ascontiguousarray(redu_w.T).astype(bf),
        "fgwT": np.ascontiguousarray((filter_gen_w / 484.0).T).astype(bf),
        "fuT": np.ascontiguousarray(fusion_w.T).astype(bf),
        "eye": np.eye(128, dtype=bf),
    }
    rb4 = np.ascontiguousarray(redu_b.reshape(MT, 128).T)
    gb4 = np.ascontiguousarray(filter_gen_b.reshape(MT, 128).T)
    fb4 = np.ascontiguousarray(fusion_b.reshape(MT, 128).T)
    in_maps = []
    for i in range(8):
        b, half = i // 2, i % 2
        if half == 0:
            rows = x[b, :, 0:33, :]
            m9 = [1, 1, 1, 1, 1, 1, 0, 0, 0]
            mgf = [1.0] * 9 + [0.0] * 9
        else:
            rows = x[b, :, 63:30:-1, :]
            m9 = [0, 0, 0, 1, 1, 1, 1, 1, 1]
            mgf = [0.0] * 9 + [1.0] * 9
        xs = np.concatenate(
            [np.zeros((CIN, 1, W), np.float32), rows], axis=1)
        blob = np.concatenate([
            rb4, gb4, fb4,
            np.tile(np.asarray(mgf, np.float32), (128, 1)),
            np.tile(np.asarray(m9, np.float32), (128, KT)),
        ], axis=1)
        assert blob.shape == (128, 174), blob.shape
        in_maps.append({
            **shared,
            "x_sh": np.ascontiguousarray(xs),
            "blob": np.ascontiguousarray(blob),
        })
    return in_maps


def kernel(x, filter_gen_w, filter_gen_b, redu_w, redu_b, fusion_w, fusion_b):
    x = np.asarray(x, np.float32)
    if "nc" not in _CACHE:
        _CACHE["nc"] = build_graph()
    nc = _CACHE["nc"]
    in_maps = _host_inputs(
        x, np.asarray(filter_gen_w, np.float32),
        np.asarray(filter_gen_b, np.float32),
        np.asarray(redu_w, np.float32), np.asarray(redu_b, np.float32),
        np.asarray(fusion_w, np.float32), np.asarray(fusion_b, np.float32))
    trace = os.environ.get("KERNEL_TRACE") == "1"
    res = run_bass_kernel_spmd(nc, in_maps, list(range(8)), trace=trace)
    if res.exec_time_ns is not None:
        print(f"HW exec time: {res.exec_time_ns} ns")
    out = np.zeros((4, C, H, W), np.float32)
    for i in range(8):
        b, half = i // 2, i % 2
        r = np.asarray(res.results[i]["out"])
        if half == 0:
            out[b, :, 0:32] = r
        else:
            out[b, :, 32:64] = r[:, ::-1, :]
    return out

